# revision 1
# baseline (speedup 1.0000x reference)
"""Bucket (block-diagonal) attention layer for Trainium2, 8 NeuronCores SPMD.

Sharding: data-parallel over batch (4) x tensor-parallel over head groups (2).
Core c = b*2 + g handles batch b, global heads [g*8, g*8+8).

Per-core math (local out dim 512 = 8 heads x 64):
  qT[dl, t] = sum_k Wq[g*512+dl, k] * x[b, t, k]  (+ bq)   [transposed layout]
  kT[dl, t] = likewise (bk dropped: constant-per-row score shifts cancel in
              softmax -- only bq enters scores via bq . k_j)
  v[t, dl]  = natural layout, with a ones-column appended per head so the
              attended matmul also produces the softmax denominator.
  scoresT[kt, qt] = matmul(lhsT=kT_head, rhs=qT_head)      (K=64)
  expT = exp(scoresT)  (no max subtraction; logits sigma ~3.3, safe in f32)
  att[qt, 0:64], den[qt] = matmul(lhsT=expT, rhs=[v_head | ones])
  y = att / den + (x_slice + bv)   [residual + bv folded on host]

All matmuls bf16 (f32 accumulate in PSUM); softmax/normalize in f32.
"""

import json
import sys

import numpy as np
import ml_dtypes

BF16 = ml_dtypes.bfloat16
FP16 = np.float16

B, S, D = 4, 4096, 1024
H, NB = 16, 32
HG = 2            # head groups (tensor parallel over heads)
NCORES = B * HG   # 8
DL = D // HG      # 512 local output dims per core
HL = H // HG      # 8 local heads
HD = D // H       # 64 head dim
BS = S // NB      # 128 bucket size
KC = D // 128     # 8 contraction chunks
NQ = 4            # token quarters processed as pipeline phases
TOKQ = S // NQ    # 1024 tokens per quarter
NBQ = TOKQ // BS  # 8 buckets per quarter
VW = 66           # per-head block width in v tiles: 64 data + 1 ones + 1 pad

_built = None     # cached (nc,) so repeated kernel() calls reuse the program


def _apply_waitfix():
    """This container's walrus accepts at most ONE sem wait per instruction.
    Post-process the BIR json: hoist extra waits onto injected wait-only
    EventSemaphore instructions just before the owning instruction."""
    import concourse.bass as bass

    if getattr(bass.Bass, "_waitfix_applied", False):
        return
    orig = bass.Bass.to_json_bytes

    def _split(m):
        n = 0
        for f in m["functions"]:
            for blk in f["blocks"]:
                out = []
                for inst in blk["instructions"]:
                    si = inst.get("sync_info")
                    if si and si.get("on_wait") and len(si["on_wait"]) > 1:
                        waits = si["on_wait"]
                        si["on_wait"] = waits[-1:]
                        for k, w in enumerate(waits[:-1]):
                            out.append({
                                "debug": inst.get("debug", 0),
                                "engine": inst["engine"],
                                "ins": [],
                                "outs": [],
                                "name": f"wfix{n}_{k}_{inst['name']}",
                                "opcode": "EventSemaphore",
                                "sync_info": {"on_update": [], "on_wait": [w]},
                            })
                        n += 1
                    out.append(inst)
                blk["instructions"] = out
        return n

    def patched(self):
        m = json.loads(orig(self))
        _split(m)
        return json.dumps(m).encode()

    bass.Bass.to_json_bytes = patched
    bass.Bass._waitfix_applied = True


def _build():
    global _built
    if _built is not None:
        return _built

    _apply_waitfix()
    import concourse.bass as bass
    import concourse.tile as tile
    from concourse import mybir
    from concourse.bass import ts

    f32 = mybir.dt.float32
    bf16 = mybir.dt.float16
    Act = mybir.ActivationFunctionType
    Alu = mybir.AluOpType

    nc = bass.Bass()
    xt = nc.dram_tensor("xt", [D, S], bf16, kind="ExternalInput")
    wq = nc.dram_tensor("wq", [D, DL], bf16, kind="ExternalInput")
    wk = nc.dram_tensor("wk", [D, DL], bf16, kind="ExternalInput")
    wv = nc.dram_tensor("wv", [D, DL], bf16, kind="ExternalInput")
    bqt = nc.dram_tensor("bq", [128, DL // 128], f32, kind="ExternalInput")
    xres = nc.dram_tensor("xres", [S, DL], f32, kind="ExternalInput")
    y = nc.dram_tensor("y", [S, DL], f32, kind="ExternalOutput")

    OD = DL // 128  # 4 out-dim partition tiles for qT/kT

    with tile.TileContext(nc) as tc:
        with (
            tc.tile_pool(name="wpool", bufs=1) as wpool,
            tc.tile_pool(name="xtp", bufs=12) as xtp,
            tc.tile_pool(name="qtp", bufs=2 * OD) as qtp,
            tc.tile_pool(name="ktp", bufs=2 * OD) as ktp,
            tc.tile_pool(name="vp", bufs=2 * NBQ) as vpool,
            tc.tile_pool(name="ep", bufs=6) as epool,
            tc.tile_pool(name="yp", bufs=3) as ypool,
            tc.tile_pool(name="xrp", bufs=4) as xrpool,
            tc.tile_pool(name="rp", bufs=8) as rpool,
            # HW constraint found empirically: each start=True matmul group
            # needs its OWN psum bank (same-bank groups corrupt for K<128 and
            # crash for mixed base partitions). 2 + 4 + 2 = 8 banks.
            tc.tile_pool(name="ps_qkv", bufs=2, space="PSUM") as ps_qkv,
            tc.tile_pool(name="ps_s", bufs=4, space="PSUM") as ps_s,
            tc.tile_pool(name="ps_a", bufs=2, space="PSUM") as ps_a,
        ):
            # --- stationary weights + bias, loaded once ---
            # wq/wk first: they gate the first projection matmuls; wv only
            # gates the v phase which runs later.
            wq_sb, wk_sb, wv_sb = [], [], []
            for lst, src, nm in ((wq_sb, wq, "wq"), (wk_sb, wk, "wk"),
                                 (wv_sb, wv, "wv")):
                for kk in range(KC):
                    t = wpool.tile([128, DL], bf16, tag=f"{nm}{kk}",
                                   name=f"{nm}{kk}")
                    nc.sync.dma_start(out=t[:], in_=src[ts(kk, 128), :])
                    lst.append(t)
            bq_sb = wpool.tile([128, OD], f32, tag="bq")
            nc.sync.dma_start(out=bq_sb[:], in_=bqt[:, :])

            for q in range(NQ):
                tok0 = q * TOKQ
                # --- load xT chunks for this quarter ---
                xt_sb = []
                for kk in range(KC):
                    t = xtp.tile([128, TOKQ], bf16, tag="xt")
                    nc.sync.dma_start(
                        out=t[:], in_=xt[ts(kk, 128), tok0:tok0 + TOKQ])
                    xt_sb.append(t)

                # --- q/k projections: psum[od-tile, 512 tok] over 8 k-chunks
                qt_sb = [qtp.tile([128, TOKQ], bf16, tag="qt", name="qt")
                         for _ in range(OD)]
                kt_sb = [ktp.tile([128, TOKQ], bf16, tag="kt", name="kt")
                         for _ in range(OD)]
                for od in range(OD):
                    for tt in range(TOKQ // 512):
                        pq = ps_qkv.tile([128, 512], f32, tag="pqkv")
                        for kk in range(KC):
                            nc.tensor.matmul(
                                pq[:], wq_sb[kk][:, ts(od, 128)],
                                xt_sb[kk][:, ts(tt, 512)],
                                start=(kk == 0), stop=(kk == KC - 1))
                        nc.scalar.activation(
                            qt_sb[od][:, ts(tt, 512)], pq[:], Act.Identity,
                            bias=bq_sb[:, od:od + 1], scale=1.0)
                        pk = ps_qkv.tile([128, 512], f32, tag="pqkv")
                        for kk in range(KC):
                            nc.tensor.matmul(
                                pk[:], wk_sb[kk][:, ts(od, 128)],
                                xt_sb[kk][:, ts(tt, 512)],
                                start=(kk == 0), stop=(kk == KC - 1))
                        nc.scalar.copy(kt_sb[od][:, ts(tt, 512)], pk[:])

                # --- v projection (natural layout), one bucket per psum ---
                v_sb = []
                for vt in range(NBQ):
                    pv = ps_qkv.tile([128, 512], f32, tag="pqkv")
                    for kk in range(KC):
                        nc.tensor.matmul(
                            pv[:], xt_sb[kk][:, ts(vt, 128)], wv_sb[kk][:],
                            start=(kk == 0), stop=(kk == KC - 1))
                    vt_sb = vpool.tile([128, HL * VW], f32, tag="v")
                    v3 = vt_sb[:].rearrange("p (h c) -> p h c", c=VW)
                    nc.vector.memset(v3[:, :, 64:66], 1.0)
                    nc.vector.tensor_copy(
                        v3[:, :, 0:64],
                        pv[:].rearrange("p (h c) -> p h c", c=HD))
                    v_sb.append(vt_sb)

                # --- attention per bucket ---
                for bk in range(NBQ):
                    col = ts(bk, BS)  # token slice within quarter
                    xr = xrpool.tile([128, DL], f32, tag="xres")
                    nc.sync.dma_start(
                        out=xr[:], in_=xres[tok0 + bk * BS:tok0 + (bk + 1) * BS, :])
                    yt = ypool.tile([128, DL], f32, tag="yt")
                    for h in range(HL):
                        od, po = h // 2, (h % 2) * 64
                        psc = ps_s.tile([128, 128], f32, tag="ps", name="ps")
                        nc.tensor.matmul(
                            psc[:],
                            kt_sb[od][po:po + 64, col],
                            qt_sb[od][po:po + 64, col],
                            start=True, stop=True)
                        ex = epool.tile([128, 128], f32, tag="expT",
                                        name="ex")
                        nc.scalar.activation(ex[:], psc[:], Act.Exp)
                        pa = ps_a.tile([128, VW], f32, tag="pa", name="pa")
                        nc.tensor.matmul(
                            pa[:], ex[:],
                            v_sb[bk][:, h * VW:(h + 1) * VW],
                            start=True, stop=True)
                        rc = rpool.tile([128, 1], f32, tag="r", name="rc")
                        nc.vector.reciprocal(rc[:], pa[:, 64:65])
                        nc.vector.scalar_tensor_tensor(
                            out=yt[:, ts(h, HD)],
                            in0=pa[:, 0:64],
                            scalar=rc[:],
                            in1=xr[:, ts(h, HD)],
                            op0=Alu.mult, op1=Alu.add)
                    nc.sync.dma_start(
                        out=y[tok0 + bk * BS:tok0 + (bk + 1) * BS, :], in_=yt[:])

    _built = nc
    return nc


def _prep_in_maps(x, Wq, bq, Wk, bk, Wv, bv):
    x = np.asarray(x, np.float32)
    Wq = np.asarray(Wq, np.float32)
    Wv = np.asarray(Wv, np.float32)
    Wk = np.asarray(Wk, np.float32)
    bq = np.asarray(bq, np.float32)
    bv = np.asarray(bv, np.float32)

    xt_b = [np.ascontiguousarray(x[b].T).astype(FP16) for b in range(B)]
    wq_g, wk_g, wv_g, bq_g = [], [], [], []
    for g in range(HG):
        sl = slice(g * DL, (g + 1) * DL)
        wq_g.append(np.ascontiguousarray(Wq[sl, :].T).astype(FP16))
        wk_g.append(np.ascontiguousarray(Wk[sl, :].T).astype(FP16))
        wv_g.append(np.ascontiguousarray(Wv[sl, :].T).astype(FP16))
        bq_g.append(np.ascontiguousarray(
            bq[sl].reshape(DL // 128, 128).T).astype(np.float32))

    in_maps = []
    for c in range(NCORES):
        b, g = c // HG, c % HG
        sl = slice(g * DL, (g + 1) * DL)
        xres = (x[b][:, sl] + bv[None, sl]).astype(np.float32)
        in_maps.append({
            "xt": xt_b[b], "wq": wq_g[g], "wk": wk_g[g], "wv": wv_g[g],
            "bq": bq_g[g], "xres": np.ascontiguousarray(xres),
        })
    return in_maps


def _gather(results):
    out = np.empty((B, S, D), np.float32)
    for c, r in enumerate(results):
        b, g = c // HG, c % HG
        out[b, :, g * DL:(g + 1) * DL] = r["y"]
    return out


def _run(inputs, trace=False, trace_cores=None):
    nc = _build()
    from concourse.bass_utils import run_bass_kernel_spmd

    in_maps = _prep_in_maps(**inputs)
    res = run_bass_kernel_spmd(
        nc, in_maps, core_ids=list(range(NCORES)), trace=trace,
        trace_cores=trace_cores)
    return _gather(res.results), res


def kernel(**inputs):
    out, _ = _run(inputs, trace=False)
    return out


def kernel_traced(trace_cores=None, **inputs):
    """For test.py: returns (output, BassKernelResults with exec_time_ns)."""
    import types
    import trn_agent_boot.trn_boot as tb

    if "antenv.axon_hooks" not in sys.modules:
        hooks = types.ModuleType("antenv.axon_hooks")
        state = [None]
        hooks.set_axon_ntff_profile_hook = lambda h: state.__setitem__(0, h)
        hooks.get_axon_ntff_profile_hook = lambda: state[0]
        sys.modules["antenv.axon_hooks"] = hooks
        hooks.set_axon_ntff_profile_hook(
            tb._ntff_profile_via_ctypes("/opt/axon/libaxon_pjrt.so"))
    return _run(inputs, trace=True, trace_cores=trace_cores)



# revision 6
# speedup vs baseline: 1.5441x; 1.5441x over previous
"""Bucket (block-diagonal) attention layer for Trainium2, 8 NeuronCores SPMD.

Sharding: data-parallel over batch (4) x tensor-parallel over head groups (2).
Core c = b*2 + g handles batch b, global heads [g*8, g*8+8).

Per-core math (local out dim 512 = 8 heads x 64):
  qT[dl, t] = sum_k Wq[g*512+dl, k] * x[b, t, k]  (+ bq)   [transposed layout]
  kT[dl, t] = likewise (bk dropped: constant-per-row score shifts cancel in
              softmax -- only bq enters scores via bq . k_j)
  v[t, dl]  = natural layout, with a ones-column appended per head so the
              attended matmul also produces the softmax denominator.
  scoresT[kt, qt] = matmul(lhsT=kT_head, rhs=qT_head)      (K=64)
  expT = exp(scoresT)  (no max subtraction; logits sigma ~3.3, safe)
  att[qt, 0:64], den[qt] = matmul(lhsT=expT, rhs=[v_head | ones])
  y = att / den + (x_slice + bv)   [residual + bv folded on host]

v2 changes vs baseline:
  - expT and v are bf16 (was f32): attended matmul runs single-pass at
    1 cycle/row instead of fp32's 4, and its LDWEIGHTS halves.
  - scores/attended matmuls for 4 heads chain into ONE psum bank via
    start=False (start=True clears the whole bank), so exp becomes one
    [128,512] activation per 4 heads and the reciprocal one strided
    [128,4] op per 4 heads -- amortizing per-instruction overhead.

All matmuls f32-accumulate in PSUM; softmax/normalize in f32.
"""

import json
import sys

import numpy as np
import ml_dtypes

BF16 = ml_dtypes.bfloat16
FP16 = np.float16

B, S, D = 4, 4096, 1024
H, NB = 16, 32
HG = 2            # head groups (tensor parallel over heads)
NCORES = B * HG   # 8
DL = D // HG      # 512 local output dims per core
HL = H // HG      # 8 local heads
HD = D // H       # 64 head dim
BS = S // NB      # 128 bucket size
KC = D // 128     # 8 contraction chunks
NQ = 4            # token quarters processed as pipeline phases
TOKQ = S // NQ    # 1024 tokens per quarter
NBQ = TOKQ // BS  # 8 buckets per quarter
VW = 66           # per-head block width in v tiles: 64 data + 1 ones + 1 pad

# Chain 4 heads' score/attended matmuls into one psum bank (start=False
# after the first), enabling batched exp/reciprocal. Fall back to
# bank-per-head if hardware disagrees with the whole-bank-clear model.
BANK_CHAIN = True

_built = None     # cached (nc,) so repeated kernel() calls reuse the program


def _apply_waitfix():
    """This container's walrus accepts at most ONE sem wait per instruction.
    Post-process the BIR json: hoist extra waits onto injected wait-only
    EventSemaphore instructions just before the owning instruction."""
    import concourse.bass as bass

    if getattr(bass.Bass, "_waitfix_applied", False):
        return
    orig = bass.Bass.to_json_bytes

    def _split(m):
        n = 0
        for f in m["functions"]:
            for blk in f["blocks"]:
                out = []
                for inst in blk["instructions"]:
                    si = inst.get("sync_info")
                    if si and si.get("on_wait") and len(si["on_wait"]) > 1:
                        waits = si["on_wait"]
                        si["on_wait"] = waits[-1:]
                        for k, w in enumerate(waits[:-1]):
                            out.append({
                                "debug": inst.get("debug", 0),
                                "engine": inst["engine"],
                                "ins": [],
                                "outs": [],
                                "name": f"wfix{n}_{k}_{inst['name']}",
                                "opcode": "EventSemaphore",
                                "sync_info": {"on_update": [], "on_wait": [w]},
                            })
                        n += 1
                    out.append(inst)
                blk["instructions"] = out
        return n

    def patched(self):
        m = json.loads(orig(self))
        _split(m)
        return json.dumps(m).encode()

    bass.Bass.to_json_bytes = patched
    bass.Bass._waitfix_applied = True


def _build():
    global _built
    if _built is not None:
        return _built

    _apply_waitfix()
    import concourse.bass as bass
    import concourse.tile as tile
    from concourse import mybir
    from concourse.bass import ts

    f32 = mybir.dt.float32
    f16 = mybir.dt.float16
    bf16 = mybir.dt.bfloat16
    Act = mybir.ActivationFunctionType
    Alu = mybir.AluOpType

    nc = bass.Bass()
    xt = nc.dram_tensor("xt", [D, S], f16, kind="ExternalInput")
    wq = nc.dram_tensor("wq", [D, DL], f16, kind="ExternalInput")
    wk = nc.dram_tensor("wk", [D, DL], f16, kind="ExternalInput")
    wv = nc.dram_tensor("wv", [D, DL], f16, kind="ExternalInput")
    bqt = nc.dram_tensor("bq", [128, DL // 128], f32, kind="ExternalInput")
    xres = nc.dram_tensor("xres", [S, DL], f32, kind="ExternalInput")
    y = nc.dram_tensor("y", [S, DL], f32, kind="ExternalOutput")

    OD = DL // 128  # 4 out-dim partition tiles for qT/kT

    with tile.TileContext(nc) as tc:
        with (
            tc.tile_pool(name="wpool", bufs=1) as wpool,
            tc.tile_pool(name="xtp", bufs=12) as xtp,
            tc.tile_pool(name="qtp", bufs=2 * OD) as qtp,
            tc.tile_pool(name="ktp", bufs=2 * OD) as ktp,
            tc.tile_pool(name="vp", bufs=2 * NBQ) as vpool,
            tc.tile_pool(name="ep", bufs=4) as epool,
            tc.tile_pool(name="yp", bufs=3) as ypool,
            tc.tile_pool(name="xrp", bufs=4) as xrpool,
            tc.tile_pool(name="rp", bufs=8) as rpool,
            tc.tile_pool(name="ps_qkv", bufs=2, space="PSUM") as ps_qkv,
            tc.tile_pool(name="ps_s", bufs=3, space="PSUM") as ps_s,
            tc.tile_pool(name="ps_a", bufs=3, space="PSUM") as ps_a,
        ):
            # --- stationary weights + bias, loaded once ---
            wq_sb, wk_sb, wv_sb = [], [], []
            for lst, src, nm in ((wq_sb, wq, "wq"), (wk_sb, wk, "wk"),
                                 (wv_sb, wv, "wv")):
                for kk in range(KC):
                    t = wpool.tile([128, DL], f16, tag=f"{nm}{kk}",
                                   name=f"{nm}{kk}")
                    nc.sync.dma_start(out=t[:], in_=src[ts(kk, 128), :])
                    lst.append(t)
            bq_sb = wpool.tile([128, OD], f32, tag="bq")
            nc.sync.dma_start(out=bq_sb[:], in_=bqt[:, :])

            for q in range(NQ):
                tok0 = q * TOKQ
                # --- load xT chunks for this quarter ---
                xt_sb = []
                for kk in range(KC):
                    t = xtp.tile([128, TOKQ], f16, tag="xt")
                    nc.sync.dma_start(
                        out=t[:], in_=xt[ts(kk, 128), tok0:tok0 + TOKQ])
                    xt_sb.append(t)

                # --- q/k projections: psum[od-tile, 512 tok] over 8 k-chunks
                qt_sb = [qtp.tile([128, TOKQ], f16, tag="qt", name="qt")
                         for _ in range(OD)]
                kt_sb = [ktp.tile([128, TOKQ], f16, tag="kt", name="kt")
                         for _ in range(OD)]
                for od in range(OD):
                    for tt in range(TOKQ // 512):
                        pq = ps_qkv.tile([128, 512], f32, tag="pqkv")
                        for kk in range(KC):
                            nc.tensor.matmul(
                                pq[:], wq_sb[kk][:, ts(od, 128)],
                                xt_sb[kk][:, ts(tt, 512)],
                                start=(kk == 0), stop=(kk == KC - 1))
                        nc.scalar.activation(
                            qt_sb[od][:, ts(tt, 512)], pq[:], Act.Identity,
                            bias=bq_sb[:, od:od + 1], scale=1.0)
                        pk = ps_qkv.tile([128, 512], f32, tag="pqkv")
                        for kk in range(KC):
                            nc.tensor.matmul(
                                pk[:], wk_sb[kk][:, ts(od, 128)],
                                xt_sb[kk][:, ts(tt, 512)],
                                start=(kk == 0), stop=(kk == KC - 1))
                        nc.scalar.copy(kt_sb[od][:, ts(tt, 512)], pk[:])

                # --- v projection (natural layout), one bucket per psum ---
                v_sb = []
                for vt in range(NBQ):
                    pv = ps_qkv.tile([128, 512], f32, tag="pqkv")
                    for kk in range(KC):
                        nc.tensor.matmul(
                            pv[:], xt_sb[kk][:, ts(vt, 128)], wv_sb[kk][:],
                            start=(kk == 0), stop=(kk == KC - 1))
                    vt_sb = vpool.tile([128, HL * VW], bf16, tag="v")
                    v3 = vt_sb[:].rearrange("p (h c) -> p h c", c=VW)
                    nc.vector.memset(v3[:, :, 64:66], 1.0)
                    nc.vector.tensor_copy(
                        v3[:, :, 0:64],
                        pv[:].rearrange("p (h c) -> p h c", c=HD))
                    v_sb.append(vt_sb)

                # --- attention per bucket ---
                for bk in range(NBQ):
                    col = ts(bk, BS)  # token slice within quarter
                    xr = xrpool.tile([128, DL], f32, tag="xres")
                    nc.sync.dma_start(
                        out=xr[:],
                        in_=xres[tok0 + bk * BS:tok0 + (bk + 1) * BS, :])
                    yt = ypool.tile([128, DL], f32, tag="yt")
                    if BANK_CHAIN:
                        # Head h sits at od-tile h%4, partitions (h//4)*64..
                        # (host permutes Wq/Wk rows to match), so each
                        # bank's 4 chained score MMs share one base
                        # partition -- mixed base partitions in one bank
                        # crash the PE. Both score groups are emitted before
                        # the attended groups so exp(g0) overlaps scores(g1)
                        # instead of stalling the PE.
                        ex_g = []
                        for g in range(2):  # 4 heads per psum bank
                            po = g * 64
                            psc = ps_s.tile([128, 512], f32, tag="ps",
                                            name="ps")
                            for hh in range(4):
                                nc.tensor.matmul(
                                    psc[:, ts(hh, 128)],
                                    kt_sb[hh][po:po + 64, col],
                                    qt_sb[hh][po:po + 64, col],
                                    start=(hh == 0), stop=(hh == 3),
                                    skip_group_check=True)
                            ex = epool.tile([128, 512], bf16, tag="ex",
                                            name="ex")
                            nc.scalar.activation(ex[:], psc[:], Act.Exp)
                            ex_g.append(ex)
                        for g in range(2):
                            ex = ex_g[g]
                            pa = ps_a.tile([128, 512], f32, tag="pa",
                                           name="pa")
                            for hh in range(4):
                                h = g * 4 + hh
                                nc.tensor.matmul(
                                    pa[:, hh * 128:hh * 128 + 66],
                                    ex[:, ts(hh, 128)],
                                    v_sb[bk][:, h * VW:(h + 1) * VW],
                                    start=(hh == 0), stop=(hh == 3),
                                    skip_group_check=True)
                            rc = rpool.tile([128, 4], f32, tag="r",
                                            name="rc")
                            pa3 = pa[:].rearrange("p (h c) -> p h c", c=128)
                            nc.vector.reciprocal(rc[:], pa3[:, :, 64])
                            for hh in range(4):
                                h = g * 4 + hh
                                nc.vector.scalar_tensor_tensor(
                                    out=yt[:, ts(h, HD)],
                                    in0=pa[:, hh * 128:hh * 128 + 64],
                                    scalar=rc[:, hh:hh + 1],
                                    in1=xr[:, ts(h, HD)],
                                    op0=Alu.mult, op1=Alu.add)
                    else:
                        for h in range(HL):
                            od, po = h % 4, (h // 4) * 64
                            psc = ps_s.tile([128, 128], f32, tag="ps",
                                            name="ps")
                            nc.tensor.matmul(
                                psc[:],
                                kt_sb[od][po:po + 64, col],
                                qt_sb[od][po:po + 64, col],
                                start=True, stop=True)
                            ex = epool.tile([128, 128], bf16, tag="ex",
                                            name="ex")
                            nc.scalar.activation(ex[:], psc[:], Act.Exp)
                            pa = ps_a.tile([128, VW], f32, tag="pa",
                                           name="pa")
                            nc.tensor.matmul(
                                pa[:], ex[:],
                                v_sb[bk][:, h * VW:(h + 1) * VW],
                                start=True, stop=True)
                            rc = rpool.tile([128, 1], f32, tag="r",
                                            name="rc")
                            nc.vector.reciprocal(rc[:], pa[:, 64:65])
                            nc.vector.scalar_tensor_tensor(
                                out=yt[:, ts(h, HD)],
                                in0=pa[:, 0:64],
                                scalar=rc[:],
                                in1=xr[:, ts(h, HD)],
                                op0=Alu.mult, op1=Alu.add)
                    nc.sync.dma_start(
                        out=y[tok0 + bk * BS:tok0 + (bk + 1) * BS, :],
                        in_=yt[:])

    _built = nc
    return nc


def _prep_in_maps(x, Wq, bq, Wk, bk, Wv, bv):
    x = np.asarray(x, np.float32)
    Wq = np.asarray(Wq, np.float32)
    Wv = np.asarray(Wv, np.float32)
    Wk = np.asarray(Wk, np.float32)
    bq = np.asarray(bq, np.float32)
    bv = np.asarray(bv, np.float32)

    xt_b = [np.ascontiguousarray(x[b].T).astype(FP16) for b in range(B)]
    # qT/kT row permutation: head h -> od-tile h%4, partitions (h//4)*64..
    # so score banks group 4 heads sharing one base partition.
    perm = np.empty(DL, np.int64)
    for h in range(HL):
        for i in range(HD):
            perm[(h % 4) * 128 + (h // 4) * 64 + i] = h * HD + i
    wq_g, wk_g, wv_g, bq_g = [], [], [], []
    for g in range(HG):
        sl = slice(g * DL, (g + 1) * DL)
        wq_g.append(np.ascontiguousarray(Wq[sl, :][perm].T).astype(FP16))
        wk_g.append(np.ascontiguousarray(Wk[sl, :][perm].T).astype(FP16))
        wv_g.append(np.ascontiguousarray(Wv[sl, :].T).astype(FP16))
        bq_g.append(np.ascontiguousarray(
            bq[sl][perm].reshape(DL // 128, 128).T).astype(np.float32))

    in_maps = []
    for c in range(NCORES):
        b, g = c // HG, c % HG
        sl = slice(g * DL, (g + 1) * DL)
        xres = (x[b][:, sl] + bv[None, sl]).astype(np.float32)
        in_maps.append({
            "xt": xt_b[b], "wq": wq_g[g], "wk": wk_g[g], "wv": wv_g[g],
            "bq": bq_g[g], "xres": np.ascontiguousarray(xres),
        })
    return in_maps


def _gather(results):
    out = np.empty((B, S, D), np.float32)
    for c, r in enumerate(results):
        b, g = c // HG, c % HG
        out[b, :, g * DL:(g + 1) * DL] = r["y"]
    return out


def _run(inputs, trace=False, trace_cores=None):
    nc = _build()
    from concourse.bass_utils import run_bass_kernel_spmd

    in_maps = _prep_in_maps(**inputs)
    res = run_bass_kernel_spmd(
        nc, in_maps, core_ids=list(range(NCORES)), trace=trace,
        trace_cores=trace_cores)
    return _gather(res.results), res


def kernel(**inputs):
    out, _ = _run(inputs, trace=False)
    return out


def kernel_traced(trace_cores=None, **inputs):
    """For test.py: returns (output, BassKernelResults with exec_time_ns)."""
    import types
    import trn_agent_boot.trn_boot as tb

    if "antenv.axon_hooks" not in sys.modules:
        hooks = types.ModuleType("antenv.axon_hooks")
        state = [None]
        hooks.set_axon_ntff_profile_hook = lambda h: state.__setitem__(0, h)
        hooks.get_axon_ntff_profile_hook = lambda: state[0]
        sys.modules["antenv.axon_hooks"] = hooks
        hooks.set_axon_ntff_profile_hook(
            tb._ntff_profile_via_ctypes("/opt/axon/libaxon_pjrt.so"))
    return _run(inputs, trace=True, trace_cores=trace_cores)


# revision 11
# speedup vs baseline: 1.5968x; 1.0341x over previous
"""Bucket (block-diagonal) attention layer for Trainium2, 8 NeuronCores SPMD.

Sharding: data-parallel over batch (4) x tensor-parallel over head groups (2).
Core c = b*2 + g handles batch b, global heads [g*8, g*8+8).

Per-core math (local out dim 512 = 8 heads x 64):
  qT[dl, t] = sum_k Wq[g*512+dl, k] * x[b, t, k]  (+ bq)   [transposed layout]
  kT[dl, t] = likewise (bk dropped: constant-per-row score shifts cancel in
              softmax -- only bq enters scores via bq . k_j)
  v[t, dl]  = natural layout, with a ones-column appended per head so the
              attended matmul also produces the softmax denominator.
  scoresT[kt, qt] = matmul(lhsT=kT_head, rhs=qT_head)      (K=64)
  expT = exp(scoresT)  (no max subtraction; logits sigma ~3.3, safe)
  att[qt, 0:64], den[qt] = matmul(lhsT=expT, rhs=[v_head | ones])
  y = att / den + (x_slice + bv)   [residual + bv folded on host]

v2 changes vs baseline:
  - expT and v are bf16 (was f32): attended matmul runs single-pass at
    1 cycle/row instead of fp32's 4, and its LDWEIGHTS halves.
  - scores/attended matmuls for 4 heads chain into ONE psum bank via
    start=False (start=True clears the whole bank), so exp becomes one
    [128,512] activation per 4 heads and the reciprocal one strided
    [128,4] op per 4 heads -- amortizing per-instruction overhead.

All matmuls f32-accumulate in PSUM; softmax/normalize in f32.
"""

import json
import sys

import numpy as np
import ml_dtypes

BF16 = ml_dtypes.bfloat16
FP16 = np.float16

B, S, D = 4, 4096, 1024
H, NB = 16, 32
HG = 2            # head groups (tensor parallel over heads)
NCORES = B * HG   # 8
DL = D // HG      # 512 local output dims per core
HL = H // HG      # 8 local heads
HD = D // H       # 64 head dim
BS = S // NB      # 128 bucket size
KC = D // 128     # 8 contraction chunks
NQ = 4            # token quarters processed as pipeline phases
TOKQ = S // NQ    # 1024 tokens per quarter
NBQ = TOKQ // BS  # 8 buckets per quarter
VW = 66           # per-head block width in v tiles: 64 data + 1 ones + 1 pad

# Chain 4 heads' score/attended matmuls into one psum bank (start=False
# after the first), enabling batched exp/reciprocal. Fall back to
# bank-per-head if hardware disagrees with the whole-bank-clear model.
BANK_CHAIN = True

_built = None     # cached (nc,) so repeated kernel() calls reuse the program


def _apply_waitfix():
    """This container's walrus accepts at most ONE sem wait per instruction.
    Post-process the BIR json: hoist extra waits onto injected wait-only
    EventSemaphore instructions just before the owning instruction."""
    import concourse.bass as bass

    if getattr(bass.Bass, "_waitfix_applied", False):
        return
    orig = bass.Bass.to_json_bytes

    def _split(m):
        n = 0
        for f in m["functions"]:
            for blk in f["blocks"]:
                out = []
                for inst in blk["instructions"]:
                    si = inst.get("sync_info")
                    if si and si.get("on_wait") and len(si["on_wait"]) > 1:
                        waits = si["on_wait"]
                        si["on_wait"] = waits[-1:]
                        for k, w in enumerate(waits[:-1]):
                            out.append({
                                "debug": inst.get("debug", 0),
                                "engine": inst["engine"],
                                "ins": [],
                                "outs": [],
                                "name": f"wfix{n}_{k}_{inst['name']}",
                                "opcode": "EventSemaphore",
                                "sync_info": {"on_update": [], "on_wait": [w]},
                            })
                        n += 1
                    out.append(inst)
                blk["instructions"] = out
        return n

    def patched(self):
        m = json.loads(orig(self))
        _split(m)
        return json.dumps(m).encode()

    bass.Bass.to_json_bytes = patched
    bass.Bass._waitfix_applied = True


def _build():
    global _built
    if _built is not None:
        return _built

    _apply_waitfix()
    import concourse.bass as bass
    import concourse.tile as tile
    from concourse import mybir
    from concourse.bass import ts

    f32 = mybir.dt.float32
    f16 = mybir.dt.float16
    bf16 = mybir.dt.bfloat16
    Act = mybir.ActivationFunctionType
    Alu = mybir.AluOpType

    nc = bass.Bass()
    xt = nc.dram_tensor("xt", [D, S], f16, kind="ExternalInput")
    wq = nc.dram_tensor("wq", [D, DL], f16, kind="ExternalInput")
    wk = nc.dram_tensor("wk", [D, DL], f16, kind="ExternalInput")
    wv = nc.dram_tensor("wv", [D, DL], f16, kind="ExternalInput")
    bqt = nc.dram_tensor("bq", [128, DL // 128], f32, kind="ExternalInput")
    xres = nc.dram_tensor("xres", [S, DL], f32, kind="ExternalInput")
    y = nc.dram_tensor("y", [S, DL], f32, kind="ExternalOutput")

    OD = DL // 128  # 4 out-dim partition tiles for qT/kT

    with tile.TileContext(nc) as tc:
        with (
            tc.tile_pool(name="wpool", bufs=1) as wpool,
            tc.tile_pool(name="xtp", bufs=12) as xtp,
            tc.tile_pool(name="qtp", bufs=2 * OD) as qtp,
            tc.tile_pool(name="ktp", bufs=2 * OD) as ktp,
            tc.tile_pool(name="vp", bufs=2 * NBQ) as vpool,
            tc.tile_pool(name="ep", bufs=4) as epool,
            tc.tile_pool(name="yp", bufs=3) as ypool,
            tc.tile_pool(name="xrp", bufs=4) as xrpool,
            tc.tile_pool(name="rp", bufs=8) as rpool,
            tc.tile_pool(name="ps_qkv", bufs=2, space="PSUM") as ps_qkv,
            tc.tile_pool(name="ps_s", bufs=3, space="PSUM") as ps_s,
            tc.tile_pool(name="ps_a", bufs=3, space="PSUM") as ps_a,
        ):
            # --- stationary weights + bias, loaded once ---
            # wq + quarter-0 xT first: they alone gate the first projection
            # group, pulling the first matmul ~10us earlier.
            def wload(lst, src, nm):
                for kk in range(KC):
                    t = wpool.tile([128, DL], f16, tag=f"{nm}{kk}",
                                   name=f"{nm}{kk}")
                    nc.sync.dma_start(out=t[:], in_=src[ts(kk, 128), :])
                    lst.append(t)

            wq_sb, wk_sb, wv_sb = [], [], []
            wload(wq_sb, wq, "wq")
            xt0_sb = []
            for kk in range(KC):
                t = xtp.tile([128, TOKQ], f16, tag="xt")
                nc.sync.dma_start(out=t[:], in_=xt[ts(kk, 128), 0:TOKQ])
                xt0_sb.append(t)
            bq_sb = wpool.tile([128, OD], f32, tag="bq")
            nc.sync.dma_start(out=bq_sb[:], in_=bqt[:, :])
            wload(wk_sb, wk, "wk")
            wload(wv_sb, wv, "wv")

            for q in range(NQ):
                tok0 = q * TOKQ
                # --- load xT chunks for this quarter ---
                if q == 0:
                    xt_sb = xt0_sb
                else:
                    xt_sb = []
                    for kk in range(KC):
                        t = xtp.tile([128, TOKQ], f16, tag="xt")
                        nc.sync.dma_start(
                            out=t[:], in_=xt[ts(kk, 128), tok0:tok0 + TOKQ])
                        xt_sb.append(t)

                # --- q/k projections: psum[od-tile, 512 tok] over 8 k-chunks
                qt_sb = [qtp.tile([128, TOKQ], f16, tag="qt", name="qt")
                         for _ in range(OD)]
                kt_sb = [ktp.tile([128, TOKQ], f16, tag="kt", name="kt")
                         for _ in range(OD)]
                for od in range(OD):
                    for tt in range(TOKQ // 512):
                        pq = ps_qkv.tile([128, 512], f32, tag="pqkv")
                        for kk in range(KC):
                            nc.tensor.matmul(
                                pq[:], wq_sb[kk][:, ts(od, 128)],
                                xt_sb[kk][:, ts(tt, 512)],
                                start=(kk == 0), stop=(kk == KC - 1))
                        nc.scalar.activation(
                            qt_sb[od][:, ts(tt, 512)], pq[:], Act.Identity,
                            bias=bq_sb[:, od:od + 1], scale=1.0)
                        pk = ps_qkv.tile([128, 512], f32, tag="pqkv")
                        for kk in range(KC):
                            nc.tensor.matmul(
                                pk[:], wk_sb[kk][:, ts(od, 128)],
                                xt_sb[kk][:, ts(tt, 512)],
                                start=(kk == 0), stop=(kk == KC - 1))
                        nc.scalar.copy(kt_sb[od][:, ts(tt, 512)], pk[:])

                # --- v projection (natural layout), one bucket per psum ---
                v_sb = []
                for vt in range(NBQ):
                    pv = ps_qkv.tile([128, 512], f32, tag="pqkv")
                    for kk in range(KC):
                        nc.tensor.matmul(
                            pv[:], xt_sb[kk][:, ts(vt, 128)], wv_sb[kk][:],
                            start=(kk == 0), stop=(kk == KC - 1))
                    vt_sb = vpool.tile([128, HL * VW], bf16, tag="v")
                    v3 = vt_sb[:].rearrange("p (h c) -> p h c", c=VW)
                    nc.vector.memset(v3[:, :, 64:66], 1.0)
                    nc.vector.tensor_copy(
                        v3[:, :, 0:64],
                        pv[:].rearrange("p (h c) -> p h c", c=HD))
                    v_sb.append(vt_sb)

                # --- attention per bucket ---
                for bk in range(NBQ):
                    col = ts(bk, BS)  # token slice within quarter
                    # xr on the idle gpsimd queue: its buffer-reuse waits
                    # must not block xt loads behind it on the sync queue.
                    xr = xrpool.tile([128, DL], f32, tag="xres")
                    nc.gpsimd.dma_start(
                        out=xr[:],
                        in_=xres[tok0 + bk * BS:tok0 + (bk + 1) * BS, :])
                    yt = ypool.tile([128, DL], f32, tag="yt")
                    if BANK_CHAIN:
                        # Head h sits at od-tile h%4, partitions (h//4)*64..
                        # (host permutes Wq/Wk rows to match), so each
                        # bank's 4 chained score MMs share one base
                        # partition -- mixed base partitions in one bank
                        # crash the PE. Both score groups are emitted before
                        # the attended groups so exp(g0) overlaps scores(g1)
                        # instead of stalling the PE.
                        # Score chains for the two banks interleave so
                        # consecutive MMs hit disjoint 64-row groups of the
                        # PE (po=0 vs po=64) and overlap in the array.
                        psc_g = [ps_s.tile([128, 512], f32, tag="ps",
                                           name="ps") for _ in range(2)]
                        for hh in range(4):
                            for g in range(2):
                                po = g * 64
                                nc.tensor.matmul(
                                    psc_g[g][:, ts(hh, 128)],
                                    kt_sb[hh][po:po + 64, col],
                                    qt_sb[hh][po:po + 64, col],
                                    start=(hh == 0), stop=(hh == 3),
                                    skip_group_check=True)
                        ex_g = []
                        for g in range(2):
                            ex = epool.tile([128, 512], bf16, tag="ex",
                                            name="ex")
                            nc.scalar.activation(ex[:], psc_g[g][:], Act.Exp)
                            ex_g.append(ex)
                        for g in range(2):
                            ex = ex_g[g]
                            pa = ps_a.tile([128, 512], f32, tag="pa",
                                           name="pa")
                            for hh in range(4):
                                h = g * 4 + hh
                                nc.tensor.matmul(
                                    pa[:, hh * 128:hh * 128 + 66],
                                    ex[:, ts(hh, 128)],
                                    v_sb[bk][:, h * VW:(h + 1) * VW],
                                    start=(hh == 0), stop=(hh == 3),
                                    skip_group_check=True)
                            rc = rpool.tile([128, 4], f32, tag="r",
                                            name="rc")
                            pa3 = pa[:].rearrange("p (h c) -> p h c", c=128)
                            nc.vector.reciprocal(rc[:], pa3[:, :, 64])
                            for hh in range(4):
                                h = g * 4 + hh
                                nc.vector.scalar_tensor_tensor(
                                    out=yt[:, ts(h, HD)],
                                    in0=pa[:, hh * 128:hh * 128 + 64],
                                    scalar=rc[:, hh:hh + 1],
                                    in1=xr[:, ts(h, HD)],
                                    op0=Alu.mult, op1=Alu.add)
                    else:
                        for h in range(HL):
                            od, po = h % 4, (h // 4) * 64
                            psc = ps_s.tile([128, 128], f32, tag="ps",
                                            name="ps")
                            nc.tensor.matmul(
                                psc[:],
                                kt_sb[od][po:po + 64, col],
                                qt_sb[od][po:po + 64, col],
                                start=True, stop=True)
                            ex = epool.tile([128, 128], bf16, tag="ex",
                                            name="ex")
                            nc.scalar.activation(ex[:], psc[:], Act.Exp)
                            pa = ps_a.tile([128, VW], f32, tag="pa",
                                           name="pa")
                            nc.tensor.matmul(
                                pa[:], ex[:],
                                v_sb[bk][:, h * VW:(h + 1) * VW],
                                start=True, stop=True)
                            rc = rpool.tile([128, 1], f32, tag="r",
                                            name="rc")
                            nc.vector.reciprocal(rc[:], pa[:, 64:65])
                            nc.vector.scalar_tensor_tensor(
                                out=yt[:, ts(h, HD)],
                                in0=pa[:, 0:64],
                                scalar=rc[:],
                                in1=xr[:, ts(h, HD)],
                                op0=Alu.mult, op1=Alu.add)
                    # y store on the gpsimd queue (vector can't issue DMAs;
                    # parking the stt-dependent store on sync would block
                    # xt loads queued behind it).
                    nc.gpsimd.dma_start(
                        out=y[tok0 + bk * BS:tok0 + (bk + 1) * BS, :],
                        in_=yt[:])

    _built = nc
    return nc


def _prep_in_maps(x, Wq, bq, Wk, bk, Wv, bv):
    x = np.asarray(x, np.float32)
    Wq = np.asarray(Wq, np.float32)
    Wv = np.asarray(Wv, np.float32)
    Wk = np.asarray(Wk, np.float32)
    bq = np.asarray(bq, np.float32)
    bv = np.asarray(bv, np.float32)

    xt_b = [np.ascontiguousarray(x[b].T).astype(FP16) for b in range(B)]
    # qT/kT row permutation: head h -> od-tile h%4, partitions (h//4)*64..
    # so score banks group 4 heads sharing one base partition.
    perm = np.empty(DL, np.int64)
    for h in range(HL):
        for i in range(HD):
            perm[(h % 4) * 128 + (h // 4) * 64 + i] = h * HD + i
    wq_g, wk_g, wv_g, bq_g = [], [], [], []
    for g in range(HG):
        sl = slice(g * DL, (g + 1) * DL)
        wq_g.append(np.ascontiguousarray(Wq[sl, :][perm].T).astype(FP16))
        wk_g.append(np.ascontiguousarray(Wk[sl, :][perm].T).astype(FP16))
        wv_g.append(np.ascontiguousarray(Wv[sl, :].T).astype(FP16))
        bq_g.append(np.ascontiguousarray(
            bq[sl][perm].reshape(DL // 128, 128).T).astype(np.float32))

    in_maps = []
    for c in range(NCORES):
        b, g = c // HG, c % HG
        sl = slice(g * DL, (g + 1) * DL)
        xres = (x[b][:, sl] + bv[None, sl]).astype(np.float32)
        in_maps.append({
            "xt": xt_b[b], "wq": wq_g[g], "wk": wk_g[g], "wv": wv_g[g],
            "bq": bq_g[g], "xres": np.ascontiguousarray(xres),
        })
    return in_maps


def _gather(results):
    out = np.empty((B, S, D), np.float32)
    for c, r in enumerate(results):
        b, g = c // HG, c % HG
        out[b, :, g * DL:(g + 1) * DL] = r["y"]
    return out


def _run(inputs, trace=False, trace_cores=None):
    nc = _build()
    from concourse.bass_utils import run_bass_kernel_spmd

    in_maps = _prep_in_maps(**inputs)
    res = run_bass_kernel_spmd(
        nc, in_maps, core_ids=list(range(NCORES)), trace=trace,
        trace_cores=trace_cores)
    return _gather(res.results), res


def kernel(**inputs):
    out, _ = _run(inputs, trace=False)
    return out


def kernel_traced(trace_cores=None, **inputs):
    """For test.py: returns (output, BassKernelResults with exec_time_ns)."""
    import types
    import trn_agent_boot.trn_boot as tb

    if "antenv.axon_hooks" not in sys.modules:
        hooks = types.ModuleType("antenv.axon_hooks")
        state = [None]
        hooks.set_axon_ntff_profile_hook = lambda h: state.__setitem__(0, h)
        hooks.get_axon_ntff_profile_hook = lambda: state[0]
        sys.modules["antenv.axon_hooks"] = hooks
        hooks.set_axon_ntff_profile_hook(
            tb._ntff_profile_via_ctypes("/opt/axon/libaxon_pjrt.so"))
    return _run(inputs, trace=True, trace_cores=trace_cores)
